# revision 23
# baseline (speedup 1.0000x reference)
"""Bahdanau additive-attention kernel for Trainium2 (8 NeuronCores).

Reference computation (per batch b):
    Wenc = enc @ W_a                      # [Te, H]
    Udec = dec @ U_a                      # [Td, H]
    scores[j, t] = sum_k V[k] * tanh(Wenc[t, k] + Udec[j, k])
    e = softmax(scores, axis=t)           # [Td, Te]
    c = e @ enc                           # [Td, H]

Shapes: B=8, Te=2048, Td=256, H=D=128.  Data-parallel: one batch per core.

Per-core structure (measured-cost-driven):
  - WencT [H=128 part, Te free] fp32 in SBUF; UdecT [H, Td] fp32.
  - One ACT instruction per decoder step j computes
    tanh(WencT + UdecT[:, j]) for all Te via the per-partition bias operand,
    output cast to fp16 (ACT is the ~490us/core bottleneck; fp16 keeps the
    PE V-reduction fast AND accurate to ~3e-4).
  - V-reduction on PE with a quadrant-replicated stationary: V broadcast to
    [H, 32] fp16, each j's output [32, 512] placed at partition base
    32*(j%4). fp16 matmuls run ~404ns vs ~1.2us for fp32 (fp32 stationary
    reload is the slow path). After 4 j's one PSUM tile holds 4 score rows
    (each replicated 32x); one DVE copy moves it to SBUF and 4 small
    SBUF->SBUF DMAs (the only partition-shifting mover) assemble
    scores[j, t] rows.
  - Softmax needs no max subtraction (|score| <= ||V||_1 ~ 13): exp with a
    constant -6 bias (cancelled exactly by normalization) keeps everything
    fp16-representable downstream.
  - c = e @ enc via fp16 PE transposes of e and fp16 K-accumulated matmuls.
"""

import os
from contextlib import ExitStack

import numpy as np

# The NTFF trace hook (antenv) is absent in this container; a stray
# BASS_TRACE in the environment would crash run_bass_kernel_spmd.
os.environ.setdefault("BASS_NEVER_TRACE", "1")

import concourse.bacc as bacc_mod
import concourse.bass as bass
import concourse.tile as tile_mod
from concourse import mybir
from concourse.bass_utils import run_bass_kernel_spmd
from concourse.masks import make_identity

B, TE, TD, H, D = 8, 2048, 256, 128, 128
P = 128
N_CHUNKS = TE // P          # 16 t-chunks of 128
N_JB = TD // P              # 2 j-blocks of 128
F32 = mybir.dt.float32
F16 = mybir.dt.float16
EXP_BIAS = -6.0             # softmax shift; cancelled by normalization


def build_program(repeat: int = 1) -> bass.Bass:
    """repeat>1 wraps the whole computation in a For_i that redoes it
    `repeat` times (identical results) — used only for timing, so the
    per-dispatch overhead can be divided out."""
    nc = bacc_mod.Bacc()

    enc_d = nc.declare_dram_parameter("encoder_out_seq", [TE, H], F32, isOutput=False)
    dec_d = nc.declare_dram_parameter("decoder_out_seq", [TD, D], F32, isOutput=False)
    W_d = nc.declare_dram_parameter("W_a", [H, H], F32, isOutput=False)
    U_d = nc.declare_dram_parameter("U_a", [D, H], F32, isOutput=False)
    V_d = nc.declare_dram_parameter("V_a", [H, 1], F32, isOutput=False)
    c_d = nc.declare_dram_parameter("c_outputs", [TD, H], F32, isOutput=True)
    e_d = nc.declare_dram_parameter("e_outputs", [TD, TE], F32, isOutput=True)

    with tile_mod.TileContext(nc) as tc, ExitStack() as rep_ctx, ExitStack() as ctx:
        if repeat > 1:
            rep_ctx.enter_context(tc.For_i(0, repeat, 1))
        consts = ctx.enter_context(tc.tile_pool(name="consts", bufs=1))
        setup = ctx.enter_context(tc.tile_pool(name="setup", bufs=1))
        tanh_pool = ctx.enter_context(tc.tile_pool(name="tanh", bufs=4))
        sc4_pool = ctx.enter_context(tc.tile_pool(name="sc4", bufs=2))
        scores_pool = ctx.enter_context(tc.tile_pool(name="scores", bufs=2))
        e_pool = ctx.enter_context(tc.tile_pool(name="e", bufs=2))
        eT_pool = ctx.enter_context(tc.tile_pool(name="eT", bufs=2))
        stat_pool = ctx.enter_context(tc.tile_pool(name="stat", bufs=4))
        cout_pool = ctx.enter_context(tc.tile_pool(name="cout", bufs=2))
        # PSUM: grp halves (2 banks x 3 bufs = 6) + tp(1) + small(1) = 8 banks
        grp_psum = ctx.enter_context(tc.tile_pool(name="grpp", bufs=3, space="PSUM"))
        tp_psum = ctx.enter_context(tc.tile_pool(name="tpp", bufs=1, space="PSUM"))
        small_psum = ctx.enter_context(tc.tile_pool(name="smallp", bufs=1, space="PSUM"))

        # ---- load inputs ----
        identity = consts.tile([P, P], F32)
        make_identity(nc, identity)
        identity16 = consts.tile([P, P], F16)
        make_identity(nc, identity16)

        enc_sb = consts.tile([P, N_CHUNKS, P], F32)  # enc[t, h] -> [t%128, t//128, h]
        nc.sync.dma_start(out=enc_sb, in_=enc_d[:, :].rearrange("(n p) h -> p n h", p=P))
        dec_sb = consts.tile([P, TD // P, D], F32)
        nc.sync.dma_start(out=dec_sb, in_=dec_d[:, :].rearrange("(n p) d -> p n d", p=P))
        W_sb = consts.tile([H, H], F32)
        nc.gpsimd.dma_start(out=W_sb, in_=W_d[:, :])
        U_sb = consts.tile([D, H], F32)
        nc.gpsimd.dma_start(out=U_sb, in_=U_d[:, :])
        V_sb = consts.tile([H, 1], F32)
        nc.gpsimd.dma_start(out=V_sb, in_=V_d[:, :])

        # V replicated across 32 columns, fp16 (stationary for the V-dot)
        Vrep16 = consts.tile([H, 32], F16)
        nc.vector.memset(Vrep16, 1.0)
        nc.vector.tensor_scalar_mul(out=Vrep16, in0=Vrep16, scalar1=V_sb)

        exp_bias_sb = consts.tile([P, 1], F32)
        nc.vector.memset(exp_bias_sb, EXP_BIAS)

        # enc cast to fp16 for the c-matmul
        enc16_sb = consts.tile([P, N_CHUNKS, P], F16)
        nc.vector.tensor_copy(enc16_sb, enc_sb)

        # ---- encT via PE transpose; WencT = W^T @ encT ----
        encT_sb = setup.tile([P, N_CHUNKS, P], F32)  # [h, t//128, t%128]
        for g in range(4):
            tp4 = tp_psum.tile([P, 4, P], F32, tag="tp", name=f"tpe{g}")
            for m in range(4):
                nc.tensor.transpose(tp4[:, m, :], enc_sb[:, 4 * g + m, :], identity)
            nc.vector.tensor_copy(encT_sb[:, 4 * g : 4 * g + 4, :], tp4)

        wencT_sb = consts.tile([H, TE], F32)  # [k, t]
        for q in range(4):
            wp = grp_psum.tile([P, 512], F32, tag="grp", name=f"wp{q}")
            nc.tensor.matmul(wp, W_sb, encT_sb[:, 4 * q : 4 * q + 4, :], start=True, stop=True)
            nc.vector.tensor_copy(wencT_sb[:, 512 * q : 512 * (q + 1)], wp)

        # ---- decT via PE transpose; UdecT = U^T @ decT ----
        decT_tp = tp_psum.tile([P, 2, P], F32, tag="tp", name="decT_tp")
        for n in range(TD // P):
            nc.tensor.transpose(decT_tp[:, n, :], dec_sb[:, n, :], identity)
        decT_sb = setup.tile([P, TD], F32)  # [d, j]
        nc.vector.tensor_copy(decT_sb, decT_tp)
        up = small_psum.tile([P, TD], F32, tag="small")
        nc.tensor.matmul(up, U_sb, decT_sb, start=True, stop=True)
        udecT_sb = consts.tile([H, TD], F32)  # [k, j]
        nc.vector.tensor_copy(udecT_sb, up)

        # ---- main loop ----
        for jb in range(N_JB):
            scores_sb = scores_pool.tile([P, TE], F32, tag="sc")  # [j, t]
            for g4 in range(P // 4):
                halves = [
                    grp_psum.tile([P, 1024], F32, tag="grp", name=f"gh{jb}_{g4 % 2}_{h}")
                    for h in range(2)
                ]
                for m in range(4):
                    jj = g4 * 4 + m
                    j = jb * P + jj
                    th = tanh_pool.tile([P, TE], F16, tag="th")
                    nc.scalar.activation(
                        out=th,
                        in_=wencT_sb,
                        func=mybir.ActivationFunctionType.Tanh,
                        bias=udecT_sb[:, j : j + 1],
                        scale=1.0,
                    )
                    for q in range(4):
                        nc.tensor.matmul(
                            halves[q // 2][32 * m : 32 * (m + 1), 512 * (q % 2) : 512 * (q % 2 + 1)],
                            Vrep16,
                            th[:, 512 * q : 512 * (q + 1)],
                            start=True,
                            stop=True,
                            tile_position=(0, 32 * m),
                        )
                # one copy per half moves 4 score rows (each replicated x32)
                sc4_sb = sc4_pool.tile([P, TE], F32, tag="sc4")
                for h in range(2):
                    nc.vector.tensor_copy(sc4_sb[:, 1024 * h : 1024 * (h + 1)], halves[h])
                # un-permute rows: partition 32*m -> scores row g4*4+m
                for m in range(4):
                    jj = g4 * 4 + m
                    nc.sync.dma_start(
                        out=scores_sb[jj : jj + 1, :],
                        in_=sc4_sb[32 * m : 32 * m + 1, :],
                    )

            # softmax over t; constant bias keeps exp small (cancels in norm);
            # accum_out yields the denominator for free
            e_sb = e_pool.tile([P, TE], F32, tag="e")
            rsum = stat_pool.tile([P, 1], F32, tag="rsum")
            nc.scalar.activation(
                out=e_sb,
                in_=scores_sb,
                func=mybir.ActivationFunctionType.Exp,
                bias=exp_bias_sb,
                accum_out=rsum,
            )
            rinv = stat_pool.tile([P, 1], F32, tag="rinv")
            nc.vector.reciprocal(rinv, rsum)
            # fp16 unnormalized e for the c-matmul (c is rescaled at the end)
            e16_sb = e_pool.tile([P, TE], F16, tag="e16")
            nc.vector.tensor_copy(e16_sb, e_sb)
            eo_sb = e_pool.tile([P, TE], F32, tag="eo")
            nc.vector.tensor_scalar_mul(out=eo_sb, in0=e_sb, scalar1=rinv)
            nc.sync.dma_start(out=e_d[jb * P : (jb + 1) * P, :], in_=eo_sb)

            # eT[t, j] via PE transposes, then c = eT.T @ enc
            eT_sb = eT_pool.tile([P, N_CHUNKS, P], F16, tag="eT")
            for g in range(4):
                tp4 = tp_psum.tile([P, 4, P], F16, tag="tp", name=f"tpq{jb}_{g}")
                for m in range(4):
                    n = 4 * g + m
                    nc.tensor.transpose(
                        tp4[:, m, :], e16_sb[:, P * n : P * (n + 1)], identity16
                    )
                nc.vector.tensor_copy(eT_sb[:, 4 * g : 4 * g + 4, :], tp4)

            cp = small_psum.tile([P, H], F32, tag="small")
            for n in range(N_CHUNKS):
                nc.tensor.matmul(
                    cp,
                    eT_sb[:, n, :],
                    enc16_sb[:, n, :],
                    start=(n == 0),
                    stop=(n == N_CHUNKS - 1),
                )
            c_sb = cout_pool.tile([P, H], F32, tag="c")
            nc.vector.tensor_scalar_mul(out=c_sb, in0=cp, scalar1=rinv)
            nc.sync.dma_start(out=c_d[jb * P : (jb + 1) * P, :], in_=c_sb)

    nc.finalize()
    return nc


_program_cache = None


def _get_program():
    global _program_cache
    if _program_cache is None:
        _program_cache = build_program()
    return _program_cache


def kernel(encoder_out_seq, decoder_out_seq, W_a, U_a, V_a):
    enc = np.ascontiguousarray(np.asarray(encoder_out_seq, dtype=np.float32))
    dec = np.ascontiguousarray(np.asarray(decoder_out_seq, dtype=np.float32))
    W = np.ascontiguousarray(np.asarray(W_a, dtype=np.float32))
    U = np.ascontiguousarray(np.asarray(U_a, dtype=np.float32))
    V = np.ascontiguousarray(np.asarray(V_a, dtype=np.float32))

    nc = _get_program()
    in_maps = [
        {
            "encoder_out_seq": enc[b],
            "decoder_out_seq": dec[b],
            "W_a": W,
            "U_a": U,
            "V_a": V,
        }
        for b in range(B)
    ]
    res = run_bass_kernel_spmd(nc, in_maps, list(range(B)))
    c_out = np.stack([res.results[b]["c_outputs"] for b in range(B)])
    e_out = np.stack([res.results[b]["e_outputs"] for b in range(B)])
    return c_out, e_out


# revision 24
# speedup vs baseline: 1.1157x; 1.1157x over previous
"""Bahdanau additive-attention kernel for Trainium2 (8 NeuronCores).

Reference computation (per batch b):
    Wenc = enc @ W_a                      # [Te, H]
    Udec = dec @ U_a                      # [Td, H]
    scores[j, t] = sum_k V[k] * tanh(Wenc[t, k] + Udec[j, k])
    e = softmax(scores, axis=t)           # [Td, Te]
    c = e @ enc                           # [Td, H]

Shapes: B=8, Te=2048, Td=256, H=D=128.  Data-parallel: one batch per core.

Per-core structure (measured-cost-driven):
  - WencT [H=128 part, Te free] fp32 in SBUF; UdecT [H, Td] fp32.
  - One ACT instruction per decoder step j computes
    tanh(WencT + UdecT[:, j]) for all Te via the per-partition bias operand,
    output cast to fp16 (ACT is the ~490us/core bottleneck; fp16 keeps the
    PE V-reduction fast AND accurate to ~3e-4).
  - V-reduction on PE with a quadrant-replicated stationary: V broadcast to
    [H, 32] fp16, each j's output [32, 512] placed at partition base
    32*(j%4). fp16 matmuls run ~404ns vs ~1.2us for fp32 (fp32 stationary
    reload is the slow path). After 4 j's one PSUM tile holds 4 score rows
    (each replicated 32x); one DVE copy moves it to SBUF and 4 small
    SBUF->SBUF DMAs (the only partition-shifting mover) assemble
    scores[j, t] rows.
  - Softmax needs no max subtraction (|score| <= ||V||_1 ~ 13): exp with a
    constant -6 bias (cancelled exactly by normalization) keeps everything
    fp16-representable downstream.
  - c = e @ enc via fp16 PE transposes of e and fp16 K-accumulated matmuls.
"""

import os
from contextlib import ExitStack

import numpy as np

# The NTFF trace hook (antenv) is absent in this container; a stray
# BASS_TRACE in the environment would crash run_bass_kernel_spmd.
os.environ.setdefault("BASS_NEVER_TRACE", "1")

import concourse.bacc as bacc_mod
import concourse.bass as bass
import concourse.tile as tile_mod
from concourse import mybir
from concourse.bass_utils import run_bass_kernel_spmd
from concourse.masks import make_identity

B, TE, TD, H, D = 8, 2048, 256, 128, 128
P = 128
N_CHUNKS = TE // P          # 16 t-chunks of 128
N_JB = TD // P              # 2 j-blocks of 128
F32 = mybir.dt.float32
F16 = mybir.dt.float16
EXP_BIAS = -6.0             # softmax shift; cancelled by normalization


def build_program(repeat: int = 1) -> bass.Bass:
    """repeat>1 wraps the whole computation in a For_i that redoes it
    `repeat` times (identical results) — used only for timing, so the
    per-dispatch overhead can be divided out."""
    nc = bacc_mod.Bacc()

    enc_d = nc.declare_dram_parameter("encoder_out_seq", [TE, H], F32, isOutput=False)
    dec_d = nc.declare_dram_parameter("decoder_out_seq", [TD, D], F32, isOutput=False)
    W_d = nc.declare_dram_parameter("W_a", [H, H], F32, isOutput=False)
    U_d = nc.declare_dram_parameter("U_a", [D, H], F32, isOutput=False)
    V_d = nc.declare_dram_parameter("V_a", [H, 1], F32, isOutput=False)
    c_d = nc.declare_dram_parameter("c_outputs", [TD, H], F32, isOutput=True)
    e_d = nc.declare_dram_parameter("e_outputs", [TD, TE], F32, isOutput=True)

    with tile_mod.TileContext(nc) as tc, ExitStack() as rep_ctx, ExitStack() as ctx:
        if repeat > 1:
            rep_ctx.enter_context(tc.For_i(0, repeat, 1))
        consts = ctx.enter_context(tc.tile_pool(name="consts", bufs=1))
        setup = ctx.enter_context(tc.tile_pool(name="setup", bufs=1))
        tanh_pool = ctx.enter_context(tc.tile_pool(name="tanh", bufs=4))
        sc4_pool = ctx.enter_context(tc.tile_pool(name="sc4", bufs=2))
        scores_pool = ctx.enter_context(tc.tile_pool(name="scores", bufs=2))
        e_pool = ctx.enter_context(tc.tile_pool(name="e", bufs=2))
        eT_pool = ctx.enter_context(tc.tile_pool(name="eT", bufs=2))
        stat_pool = ctx.enter_context(tc.tile_pool(name="stat", bufs=4))
        cout_pool = ctx.enter_context(tc.tile_pool(name="cout", bufs=2))
        # PSUM: grp halves (2 banks x 2 bufs = 4) + tp(1) + small(2) = 7 banks
        grp_psum = ctx.enter_context(tc.tile_pool(name="grpp", bufs=2, space="PSUM"))
        tp_psum = ctx.enter_context(tc.tile_pool(name="tpp", bufs=1, space="PSUM"))
        small_psum = ctx.enter_context(tc.tile_pool(name="smallp", bufs=2, space="PSUM"))

        # ---- load inputs ----
        identity = consts.tile([P, P], F32)
        make_identity(nc, identity)
        identity16 = consts.tile([P, P], F16)
        make_identity(nc, identity16)

        enc_sb = consts.tile([P, N_CHUNKS, P], F32)  # enc[t, h] -> [t%128, t//128, h]
        nc.sync.dma_start(out=enc_sb, in_=enc_d[:, :].rearrange("(n p) h -> p n h", p=P))
        dec_sb = consts.tile([P, TD // P, D], F32)
        nc.sync.dma_start(out=dec_sb, in_=dec_d[:, :].rearrange("(n p) d -> p n d", p=P))
        W_sb = consts.tile([H, H], F32)
        nc.gpsimd.dma_start(out=W_sb, in_=W_d[:, :])
        U_sb = consts.tile([D, H], F32)
        nc.gpsimd.dma_start(out=U_sb, in_=U_d[:, :])
        V_sb = consts.tile([H, 1], F32)
        nc.gpsimd.dma_start(out=V_sb, in_=V_d[:, :])

        # V replicated across 32 columns, fp16 (stationary for the V-dot)
        Vrep16 = consts.tile([H, 32], F16)
        nc.vector.memset(Vrep16, 1.0)
        nc.vector.tensor_scalar_mul(out=Vrep16, in0=Vrep16, scalar1=V_sb)

        exp_bias_sb = consts.tile([P, 1], F32)
        nc.vector.memset(exp_bias_sb, EXP_BIAS)

        # enc cast to fp16 for the c-matmul
        enc16_sb = consts.tile([P, N_CHUNKS, P], F16)
        nc.vector.tensor_copy(enc16_sb, enc_sb)

        # ---- encT via PE transpose; WencT = W^T @ encT ----
        encT_sb = setup.tile([P, N_CHUNKS, P], F32)  # [h, t//128, t%128]
        for g in range(4):
            tp4 = tp_psum.tile([P, 4, P], F32, tag="tp", name=f"tpe{g}")
            for m in range(4):
                nc.tensor.transpose(tp4[:, m, :], enc_sb[:, 4 * g + m, :], identity)
            nc.vector.tensor_copy(encT_sb[:, 4 * g : 4 * g + 4, :], tp4)

        wencT_sb = consts.tile([H, TE], F32)  # [k, t]
        for q in range(4):
            wp = grp_psum.tile([P, 512], F32, tag="grp", name=f"wp{q}")
            nc.tensor.matmul(wp, W_sb, encT_sb[:, 4 * q : 4 * q + 4, :], start=True, stop=True)
            nc.vector.tensor_copy(wencT_sb[:, 512 * q : 512 * (q + 1)], wp)

        # ---- decT via PE transpose; UdecT = U^T @ decT ----
        decT_tp = tp_psum.tile([P, 2, P], F32, tag="tp", name="decT_tp")
        for n in range(TD // P):
            nc.tensor.transpose(decT_tp[:, n, :], dec_sb[:, n, :], identity)
        decT_sb = setup.tile([P, TD], F32)  # [d, j]
        nc.vector.tensor_copy(decT_sb, decT_tp)
        up = small_psum.tile([P, TD], F32, tag="small")
        nc.tensor.matmul(up, U_sb, decT_sb, start=True, stop=True)
        udecT_sb = consts.tile([H, TD], F32)  # [k, j]
        nc.vector.tensor_copy(udecT_sb, up)

        # ---- main loop ----
        for jb in range(N_JB):
            scores_sb = scores_pool.tile([P, TE], F32, tag="sc")  # [j, t]
            for g4 in range(P // 4):
                halves = [
                    grp_psum.tile([P, 1024], F32, tag="grp", name=f"gh{jb}_{g4 % 2}_{h}")
                    for h in range(2)
                ]
                for m in range(4):
                    jj = g4 * 4 + m
                    j = jb * P + jj
                    th = tanh_pool.tile([P, TE], F16, tag="th")
                    nc.scalar.activation(
                        out=th,
                        in_=wencT_sb,
                        func=mybir.ActivationFunctionType.Tanh,
                        bias=udecT_sb[:, j : j + 1],
                        scale=1.0,
                    )
                    for q in range(4):
                        nc.tensor.matmul(
                            halves[q // 2][32 * m : 32 * (m + 1), 512 * (q % 2) : 512 * (q % 2 + 1)],
                            Vrep16,
                            th[:, 512 * q : 512 * (q + 1)],
                            start=True,
                            stop=True,
                            tile_position=(0, 32 * m),
                        )
                # one copy per half moves 4 score rows (each replicated x32)
                sc4_sb = sc4_pool.tile([P, TE], F32, tag="sc4")
                for h in range(2):
                    nc.vector.tensor_copy(sc4_sb[:, 1024 * h : 1024 * (h + 1)], halves[h])
                # un-permute rows: partition 32*m -> scores row g4*4+m
                for m in range(4):
                    jj = g4 * 4 + m
                    nc.sync.dma_start(
                        out=scores_sb[jj : jj + 1, :],
                        in_=sc4_sb[32 * m : 32 * m + 1, :],
                    )

            # softmax over t; constant bias keeps exp small (cancels in norm);
            # accum_out yields the denominator for free
            e_sb = e_pool.tile([P, TE], F32, tag="e")
            rsum = stat_pool.tile([P, 1], F32, tag="rsum")
            nc.scalar.activation(
                out=e_sb,
                in_=scores_sb,
                func=mybir.ActivationFunctionType.Exp,
                bias=exp_bias_sb,
                accum_out=rsum,
            )
            rinv = stat_pool.tile([P, 1], F32, tag="rinv")
            nc.vector.reciprocal(rinv, rsum)
            # fp16 unnormalized e for the c-matmul (c is rescaled at the end)
            e16_sb = e_pool.tile([P, TE], F16, tag="e16")
            nc.vector.tensor_copy(e16_sb, e_sb)
            eo_sb = e_pool.tile([P, TE], F32, tag="eo")
            nc.vector.tensor_scalar_mul(out=eo_sb, in0=e_sb, scalar1=rinv)
            nc.sync.dma_start(out=e_d[jb * P : (jb + 1) * P, :], in_=eo_sb)

            # eT[t, j] via PE transposes, then c = eT.T @ enc
            eT_sb = eT_pool.tile([P, N_CHUNKS, P], F16, tag="eT")
            for g in range(4):
                tp4 = tp_psum.tile([P, 4, P], F16, tag="tp", name=f"tpq{jb}_{g}")
                for m in range(4):
                    n = 4 * g + m
                    nc.tensor.transpose(
                        tp4[:, m, :], e16_sb[:, P * n : P * (n + 1)], identity16
                    )
                nc.vector.tensor_copy(eT_sb[:, 4 * g : 4 * g + 4, :], tp4)

            cp = small_psum.tile([P, H], F32, tag="small")
            for n in range(N_CHUNKS):
                nc.tensor.matmul(
                    cp,
                    eT_sb[:, n, :],
                    enc16_sb[:, n, :],
                    start=(n == 0),
                    stop=(n == N_CHUNKS - 1),
                )
            c_sb = cout_pool.tile([P, H], F32, tag="c")
            nc.vector.tensor_scalar_mul(out=c_sb, in0=cp, scalar1=rinv)
            nc.sync.dma_start(out=c_d[jb * P : (jb + 1) * P, :], in_=c_sb)

    nc.finalize()
    return nc


_program_cache = None


def _get_program():
    global _program_cache
    if _program_cache is None:
        _program_cache = build_program()
    return _program_cache


def kernel(encoder_out_seq, decoder_out_seq, W_a, U_a, V_a):
    enc = np.ascontiguousarray(np.asarray(encoder_out_seq, dtype=np.float32))
    dec = np.ascontiguousarray(np.asarray(decoder_out_seq, dtype=np.float32))
    W = np.ascontiguousarray(np.asarray(W_a, dtype=np.float32))
    U = np.ascontiguousarray(np.asarray(U_a, dtype=np.float32))
    V = np.ascontiguousarray(np.asarray(V_a, dtype=np.float32))

    nc = _get_program()
    in_maps = [
        {
            "encoder_out_seq": enc[b],
            "decoder_out_seq": dec[b],
            "W_a": W,
            "U_a": U,
            "V_a": V,
        }
        for b in range(B)
    ]
    res = run_bass_kernel_spmd(nc, in_maps, list(range(B)))
    c_out = np.stack([res.results[b]["c_outputs"] for b in range(B)])
    e_out = np.stack([res.results[b]["e_outputs"] for b in range(B)])
    return c_out, e_out
